# revision 1
# baseline (speedup 1.0000x reference)
"""InfoNCE lower-bound kernel for 8 Trainium2 NeuronCores.

Math (reference):
  hx = x @ W1x.T ; hy = y @ W1y.T            [N, H]
  z_ij = relu(hx[j] + hy[i] + b1) . w2       (logit WITHOUT b2)
  T1[i,j] = softplus(z_ij + b2)
  T0[i]   = T1[i,i]
  lse[i]  = log(sum_j exp(T1[i,j]))
  out     = mean(T0) - (mean(lse) - log N)

Key identity used on-device:  exp(softplus(v)) = 1 + e^v, so
  lse[i] = log(N + sum_j exp(z_ij + b2))
which avoids materializing softplus over the [N, N] grid.

Sharding: data-parallel over i (rows of the pair grid). Each of the 8
cores gets 64 rows (its slice of y), x and the MLP params replicated.
Per-core partial sums of T0 and lse are combined on the host.
"""

import math

import numpy as np

N = 512
XD = 768
YD = 768
H = 300
NCORES = 8
ISH = N // NCORES  # 64 rows per core
KD = XD // 128     # 6 contraction tiles of 128
HT = 3             # h tiles: 128, 128, 44
HSZ = [128, 128, H - 256]

_CACHE = {}
TRACE = False
LAST_RESULTS = None


def _build_module():
    import concourse.bacc as bacc
    import concourse.mybir as mybir
    from concourse.tile import TileContext

    f32 = mybir.dt.float32
    AF = mybir.ActivationFunctionType
    ALU = mybir.AluOpType
    AX = mybir.AxisListType

    nc = bacc.Bacc("TRN2", target_bir_lowering=False, debug=False)

    # Per-core inputs (SPMD: same shapes, different data for yT/xTd).
    xT = nc.dram_tensor("xT", [XD, N], f32, kind="ExternalInput")        # x^T
    w1xT = nc.dram_tensor("w1xT", [XD, H], f32, kind="ExternalInput")    # W1x^T
    w1yT = nc.dram_tensor("w1yT", [YD, H], f32, kind="ExternalInput")    # W1y^T
    yT = nc.dram_tensor("yT", [YD, ISH], f32, kind="ExternalInput")      # y-slice^T
    xTd = nc.dram_tensor("xTd", [XD, ISH], f32, kind="ExternalInput")    # x-slice^T (diag)
    b1p = nc.dram_tensor("b1p", [128, HT], f32, kind="ExternalInput")    # b1 packed
    w2p = nc.dram_tensor("w2p", [128, HT], f32, kind="ExternalInput")    # w2 packed
    b2r = nc.dram_tensor("b2r", [128, 1], f32, kind="ExternalInput")     # b2 replicated
    out = nc.dram_tensor("out", [1, 2], f32, kind="ExternalOutput")      # [t0_sum, lse_sum]

    with TileContext(nc) as tc:
        cpool = tc.alloc_tile_pool(name="consts", bufs=1)
        rpool = tc.alloc_tile_pool(name="work", bufs=6)
        tpool = tc.alloc_tile_pool(name="tail", bufs=1)
        pp_pre = tc.alloc_tile_pool(name="pp_pre", bufs=2, space="PSUM")
        pp_z = tc.alloc_tile_pool(name="pp_z", bufs=3, space="PSUM")
        pp_misc = tc.alloc_tile_pool(name="pp_misc", bufs=1, space="PSUM")

        # ---- load constants / inputs into SBUF ----
        xt_sb = cpool.tile([128, KD * N], f32, tag="xt")
        w1x_sb = cpool.tile([128, KD * H], f32, tag="w1x")
        w1y_sb = cpool.tile([128, KD * H], f32, tag="w1y")
        yt_sb = cpool.tile([128, KD * ISH], f32, tag="yt")
        xtd_sb = cpool.tile([128, KD * ISH], f32, tag="xtd")
        b1_sb = cpool.tile([128, HT], f32, tag="b1")
        w2_sb = cpool.tile([128, HT], f32, tag="w2")
        b2_sb = cpool.tile([128, 1], f32, tag="b2")

        for k in range(KD):
            nc.sync.dma_start(xt_sb[:, k * N:(k + 1) * N], xT[k * 128:(k + 1) * 128, :])
            nc.sync.dma_start(w1x_sb[:, k * H:(k + 1) * H], w1xT[k * 128:(k + 1) * 128, :])
            nc.sync.dma_start(w1y_sb[:, k * H:(k + 1) * H], w1yT[k * 128:(k + 1) * 128, :])
            nc.sync.dma_start(yt_sb[:, k * ISH:(k + 1) * ISH], yT[k * 128:(k + 1) * 128, :])
            nc.sync.dma_start(xtd_sb[:, k * ISH:(k + 1) * ISH], xTd[k * 128:(k + 1) * 128, :])
        nc.sync.dma_start(b1_sb[:], b1p[:])
        nc.sync.dma_start(w2_sb[:], w2p[:])
        nc.sync.dma_start(b2_sb[:], b2r[:])

        # ---- precompute hxT(+b1), hyT, hxdT on device ----
        hxb_sb = cpool.tile([128, HT * N], f32, tag="hxb")    # relu-arg x part (+b1)
        hy_sb = cpool.tile([128, HT * ISH], f32, tag="hy")    # y part
        hxd_sb = cpool.tile([128, HT * ISH], f32, tag="hxd")  # diag x part (+b1)
        nc.vector.memset(hxb_sb[:, 2 * N:3 * N], 0.0)
        nc.vector.memset(hy_sb[:, 2 * ISH:3 * ISH], 0.0)
        nc.vector.memset(hxd_sb[:, 2 * ISH:3 * ISH], 0.0)

        for t in range(HT):
            hs = HSZ[t]
            ps = pp_pre.tile([128, N], f32, tag="pre")
            for k in range(KD):
                nc.tensor.matmul(
                    ps[0:hs, :],
                    lhsT=w1x_sb[:, k * H + 128 * t: k * H + 128 * t + hs],
                    rhs=xt_sb[:, k * N:(k + 1) * N],
                    start=(k == 0), stop=(k == KD - 1),
                )
            nc.scalar.activation(
                hxb_sb[0:hs, t * N:(t + 1) * N], ps[0:hs, :],
                AF.Identity, bias=b1_sb[0:hs, t:t + 1],
            )

        for t in range(HT):
            hs = HSZ[t]
            psy = pp_pre.tile([128, ISH], f32, tag="pre")
            for k in range(KD):
                nc.tensor.matmul(
                    psy[0:hs, :],
                    lhsT=w1y_sb[:, k * H + 128 * t: k * H + 128 * t + hs],
                    rhs=yt_sb[:, k * ISH:(k + 1) * ISH],
                    start=(k == 0), stop=(k == KD - 1),
                )
            nc.vector.tensor_copy(hy_sb[0:hs, t * ISH:(t + 1) * ISH], psy[0:hs, :])

        for t in range(HT):
            hs = HSZ[t]
            psd = pp_pre.tile([128, ISH], f32, tag="pre")
            for k in range(KD):
                nc.tensor.matmul(
                    psd[0:hs, :],
                    lhsT=w1x_sb[:, k * H + 128 * t: k * H + 128 * t + hs],
                    rhs=xtd_sb[:, k * ISH:(k + 1) * ISH],
                    start=(k == 0), stop=(k == KD - 1),
                )
            nc.scalar.activation(
                hxd_sb[0:hs, t * ISH:(t + 1) * ISH], psd[0:hs, :],
                AF.Identity, bias=b1_sb[0:hs, t:t + 1],
            )

        # ---- main loop: z rows via relu + matvec, 4 rows per PSUM bank ----
        zrows = cpool.tile([ISH, N], f32, tag="zrows")
        for g in range(ISH // 4):
            zp = pp_z.tile([128, N], f32, tag="zp")
            for k4 in range(4):
                i = 4 * g + k4
                for t in range(HT):
                    r = rpool.tile([128, N], f32, tag="r")
                    col = hy_sb[:, t * ISH + i: t * ISH + i + 1]
                    src = hxb_sb[:, t * N:(t + 1) * N]
                    if t == 1:
                        nc.scalar.activation(r[:], src, AF.Relu, bias=col)
                    else:
                        nc.vector.tensor_scalar(r[:], src, col, 0.0, ALU.add, ALU.max)
                    nc.tensor.matmul(
                        zp[32 * k4:32 * k4 + 1, :],
                        lhsT=w2_sb[:, t:t + 1], rhs=r[:],
                        start=(t == 0), stop=(t == HT - 1),
                        tile_position=(0, 32 * k4),
                    )
            zst = rpool.tile([128, N], f32, tag="zst")
            if g % 2 == 0:
                nc.vector.tensor_copy(zst[:], zp[:])
            else:
                nc.scalar.copy(zst[:], zp[:])
            zst_rows = zst[:].rearrange("(a b) f -> a b f", b=32)[:, 0, :]
            nc.sync.dma_start(zrows[4 * g:4 * g + 4, :], zst_rows)

        # ---- tail: lse partial ----
        ee = tpool.tile([ISH, N], f32, tag="ee")
        sexp = tpool.tile([ISH, 1], f32, tag="sexp")
        nc.scalar.activation(ee[:], zrows[:], AF.Exp, bias=b2_sb[0:ISH, 0:1])
        nc.vector.tensor_reduce(sexp[:], ee[:], axis=AX.X, op=ALU.add)
        lsev = tpool.tile([ISH, 1], f32, tag="lsev")
        nconst = tpool.tile([ISH, 1], f32, tag="nconst")
        nc.vector.memset(nconst[:], float(N))
        nc.scalar.activation(lsev[:], sexp[:], AF.Ln, bias=nconst[0:ISH, 0:1])
        onesv = tpool.tile([ISH, 1], f32, tag="ones")
        nc.vector.memset(onesv[:], 1.0)
        lsum_ps = pp_misc.tile([128, 1], f32, tag="lsum")
        nc.tensor.matmul(
            lsum_ps[0:1, 0:1], lhsT=onesv[0:ISH, 0:1], rhs=lsev[0:ISH, 0:1],
            start=True, stop=True,
        )

        # ---- tail: T0 partial from diagonal ----
        dps = pp_misc.tile([128, ISH], f32, tag="dps")
        for t in range(HT):
            dsum = tpool.tile([128, ISH], f32, tag="dsum")
            nc.vector.tensor_add(
                dsum[:], hxd_sb[:, t * ISH:(t + 1) * ISH], hy_sb[:, t * ISH:(t + 1) * ISH]
            )
            dr = tpool.tile([128, ISH], f32, tag="dr")
            nc.vector.tensor_scalar(dr[:], dsum[:], 0.0, None, ALU.max)
            nc.tensor.matmul(
                dps[0:1, :], lhsT=w2_sb[:, t:t + 1], rhs=dr[:],
                start=(t == 0), stop=(t == HT - 1),
            )
        ed = tpool.tile([1, ISH], f32, tag="ed")
        nc.scalar.activation(ed[:], dps[0:1, :], AF.Exp, bias=b2_sb[0:1, 0:1])
        t0v = tpool.tile([1, ISH], f32, tag="t0v")
        nc.scalar.activation(t0v[:], ed[:], AF.Ln, bias=onesv[0:1, 0:1])

        final = tpool.tile([1, 2], f32, tag="final")
        nc.vector.tensor_reduce(final[0:1, 0:1], t0v[0:1, :], axis=AX.X, op=ALU.add)
        nc.scalar.copy(final[0:1, 1:2], lsum_ps[0:1, 0:1])
        nc.sync.dma_start(out[0:1, :], final[0:1, :])

        for p in (pp_misc, pp_z, pp_pre, tpool, rpool, cpool):
            p.release()

    nc.finalize()
    return nc


def _get_module():
    if "nc" not in _CACHE:
        _CACHE["nc"] = _build_module()
    return _CACHE["nc"]


def kernel(**inputs) -> np.ndarray:
    from concourse.bass_utils import run_bass_kernel_spmd

    x = np.ascontiguousarray(np.asarray(inputs["x_samples"], dtype=np.float32))
    y = np.ascontiguousarray(np.asarray(inputs["y_samples"], dtype=np.float32))
    W1 = np.asarray(inputs["W1"], dtype=np.float32)
    b1 = np.asarray(inputs["b1"], dtype=np.float32).reshape(H)
    W2 = np.asarray(inputs["W2"], dtype=np.float32)
    b2 = float(np.asarray(inputs["b2"], dtype=np.float32).reshape(1)[0])

    xT = np.ascontiguousarray(x.T)                      # [768, 512]
    w1xT = np.ascontiguousarray(W1[:, :XD].T)           # [768, 300]
    w1yT = np.ascontiguousarray(W1[:, XD:].T)           # [768, 300]

    b1p = np.zeros((128, HT), np.float32)
    w2p = np.zeros((128, HT), np.float32)
    w2 = W2.reshape(H)
    for t in range(HT):
        hs = HSZ[t]
        b1p[:hs, t] = b1[128 * t:128 * t + hs]
        w2p[:hs, t] = w2[128 * t:128 * t + hs]
    b2r = np.full((128, 1), b2, np.float32)

    in_maps = []
    for c in range(NCORES):
        sl = slice(c * ISH, (c + 1) * ISH)
        in_maps.append({
            "xT": xT,
            "w1xT": w1xT,
            "w1yT": w1yT,
            "yT": np.ascontiguousarray(y[sl].T),        # [768, 64]
            "xTd": np.ascontiguousarray(x[sl].T),       # [768, 64]
            "b1p": b1p,
            "w2p": w2p,
            "b2r": b2r,
        })

    nc = _get_module()
    res = run_bass_kernel_spmd(
        nc, in_maps, core_ids=list(range(NCORES)), trace=TRACE
    )
    global LAST_RESULTS
    LAST_RESULTS = res
    t0_sum = 0.0
    lse_sum = 0.0
    for r in res.results:
        o = r["out"]
        t0_sum += float(o[0, 0])
        lse_sum += float(o[0, 1])
    val = t0_sum / N - (lse_sum / N - math.log(N))
    return np.float32(val)



# revision 8
# speedup vs baseline: 1.6909x; 1.6909x over previous
"""InfoNCE lower-bound kernel for 8 Trainium2 NeuronCores (v2).

Math (reference):
  hx = x @ W1x.T ; hy = y @ W1y.T            [N, H]
  z_ij = relu(hx[j] + hy[i] + b1) . w2       (logit WITHOUT b2)
  T1[i,j] = softplus(z_ij + b2)
  T0[i]   = T1[i,i]
  lse[i]  = log(sum_j exp(T1[i,j])) = log(N + sum_j exp(z_ij + b2))
  out     = mean(T0) - (mean(lse) - log N)

Layout (per core, 64 i-rows):
  * bf16 on all wide paths; fp32 PSUM accumulation.
  * H=300 split into chunks (64,64,64,64,44). i-rows processed in PAIRS:
    one [128, 512] relu tile holds chunk rows for i (partitions 0..63)
    and i+1 (partitions 64..127), built by one DVE/Act op from a
    row-duplicated hxb copy plus a paired per-partition bias column.
  * w2 contraction = m=2 block-diagonal matvec (lhsT [128,2]) on the PE,
    column-tiled 4 ways (tile_position (0,32g)); pair p accumulates into
    PSUM rows {32g, 32g+1} of bank p//4 -- all 64 z-rows in flight.
  * Banks are zeroed by an all-zeros matmul first, so untouched
    partitions are exactly 0 (NaN-safe masking later).
  * Per-bank drain (overlapped): Act Exp of the 8 diag columns, then Act
    Exp of the full bank with accum_out row-sums.
  * x columns are rotated by 64*core so the T1 diagonal of row i lands
    at column i locally; T0 is read out of the grid itself.
"""

import math

import numpy as np
import ml_dtypes

N = 512
XD = 768
YD = 768
H = 300
NCORES = 8
ISH = N // NCORES   # 64 rows per core
KD = XD // 128      # 6 contraction tiles of 128
NPAIR = ISH // 2    # 32 pairs
CH = [64, 64, 64, 64, 44]        # h-chunk sizes
CHOFF = [0, 64, 128, 192, 256]   # h offset of each chunk
CORDER = [4, 0, 1, 2, 3]         # chunk issue order (res first, stop on c=3)
BF16 = ml_dtypes.bfloat16

_CACHE = {}
TRACE = False
LAST_RESULTS = None


def _build_module():
    import concourse.bacc as bacc
    import concourse.mybir as mybir
    from concourse.tile import TileContext

    f32 = mybir.dt.float32
    bf16 = mybir.dt.bfloat16
    AF = mybir.ActivationFunctionType
    ALU = mybir.AluOpType
    AX = mybir.AxisListType

    nc = bacc.Bacc("TRN2", target_bir_lowering=False, debug=False)

    xT = nc.dram_tensor("xT", [XD, N], bf16, kind="ExternalInput")      # x^T, cols rotated by 64*core
    w1xT = nc.dram_tensor("w1xT", [XD, H], bf16, kind="ExternalInput")
    w1yT = nc.dram_tensor("w1yT", [YD, H], bf16, kind="ExternalInput")
    yT = nc.dram_tensor("yT", [YD, ISH], bf16, kind="ExternalInput")    # y-slice^T
    b1p = nc.dram_tensor("b1p", [128, 3], f32, kind="ExternalInput")    # b1 packed by h-tile
    w2pk = nc.dram_tensor("w2pk", [128, 10], bf16, kind="ExternalInput")
    b2r = nc.dram_tensor("b2r", [128, 1], f32, kind="ExternalInput")
    dmask = nc.dram_tensor("dmask", [128, 64], f32, kind="ExternalInput")
    pmask = nc.dram_tensor("pmask", [128, 1], f32, kind="ExternalInput")
    out = nc.dram_tensor("out", [1, 2], f32, kind="ExternalOutput")     # [t0_sum, lse_sum]

    with TileContext(nc) as tc:
        cpool = tc.alloc_tile_pool(name="consts", bufs=1)
        rpool = tc.alloc_tile_pool(name="rtiles", bufs=16)
        epool = tc.alloc_tile_pool(name="escratch", bufs=2)
        tpool = tc.alloc_tile_pool(name="tail", bufs=1)
        pp_pre = tc.alloc_tile_pool(name="pp_pre", bufs=2, space="PSUM")
        pp_hy = tc.alloc_tile_pool(name="pp_hy", bufs=2, space="PSUM")

        # ---- persistent SBUF tiles ----
        xt_sb = cpool.tile([128, KD * N], bf16, tag="xt")
        w1x_sb = cpool.tile([128, KD * H], bf16, tag="w1x")
        w1y_sb = cpool.tile([128, KD * H], bf16, tag="w1y")
        yt_sb = cpool.tile([128, KD * ISH], bf16, tag="yt")
        b1_sb = cpool.tile([128, 3], f32, tag="b1")
        w2_sb = cpool.tile([128, 10], bf16, tag="w2")
        b2_sb = cpool.tile([128, 1], f32, tag="b2")
        dmask_sb = cpool.tile([128, 64], f32, tag="dmask")
        pmask_sb = cpool.tile([128, 1], f32, tag="pmask")
        hyb_sb = cpool.tile([128, 3 * ISH], f32, tag="hyb")    # hy + b1, per h-tile
        hyp_sb = cpool.tile([128, 5 * NPAIR], f32, tag="hyp")  # paired bias cols
        hxd_sb = cpool.tile([128, 5 * N], bf16, tag="hxd")     # row-duplicated hxb chunks
        dee_sb = cpool.tile([128, 64], f32, tag="dee")         # e^(z+b2) diag cols per bank
        acc_sb = cpool.tile([128, 8], f32, tag="acc")          # row sums per bank
        zs_sb = cpool.tile([128, N], bf16, tag="zs")           # zero scratch

        # ---- DMA inputs ----
        nc.sync.dma_start(b1_sb[:], b1p[:])
        nc.sync.dma_start(w2_sb[:], w2pk[:])
        nc.sync.dma_start(b2_sb[:], b2r[:])
        nc.sync.dma_start(dmask_sb[:], dmask[:])
        nc.sync.dma_start(pmask_sb[:], pmask[:])
        for k in range(KD):
            nc.sync.dma_start(w1y_sb[:, k * H:(k + 1) * H], w1yT[k * 128:(k + 1) * 128, :])
            nc.sync.dma_start(yt_sb[:, k * ISH:(k + 1) * ISH], yT[k * 128:(k + 1) * 128, :])
        for k in range(KD):
            nc.sync.dma_start(w1x_sb[:, k * H:(k + 1) * H], w1xT[k * 128:(k + 1) * 128, :])
            nc.sync.dma_start(xt_sb[:, k * N:(k + 1) * N], xT[k * 128:(k + 1) * 128, :])

        # ---- PE warmup (HAM) on zeroed scratch during the DMA window ----
        nc.vector.memset(zs_sb[:], 0.0)
        for w in range(24):
            wps = pp_pre.tile([128, N], f32, tag="pre")
            nc.tensor.matmul(
                wps[0:128, 0:128], lhsT=zs_sb[:, 0:128], rhs=zs_sb[:, 0:128],
                start=True, stop=True,
            )

        # ---- precompute hy' = y @ W1y^T + b1 (fp32, 3 h-tiles) ----
        HT_SZ = [128, 128, 44]
        for t in range(3):
            hs = HT_SZ[t]
            psy = pp_hy.tile([128, ISH], f32, tag="hy")
            for k in range(KD):
                nc.tensor.matmul(
                    psy[0:hs, :],
                    lhsT=w1y_sb[:, k * H + 128 * t: k * H + 128 * t + hs],
                    rhs=yt_sb[:, k * ISH:(k + 1) * ISH],
                    start=(k == 0), stop=(k == KD - 1),
                )
            nc.vector.tensor_scalar(
                hyb_sb[0:hs, t * ISH:(t + 1) * ISH], psy[0:hs, :],
                b1_sb[0:hs, t:t + 1], None, ALU.add,
            )

        # paired bias columns: hyp[0:cs, 5p+c] = hy'[chunk c, 2p],
        #                      hyp[cs:2cs, 5p+c] = hy'[chunk c, 2p+1]
        hyb_v = hyb_sb[:].rearrange("p (t i) -> p t i", t=3)
        hyp_v = hyp_sb[:].rearrange("p (q c) -> p q c", c=5)
        for c in range(5):
            cs = CH[c]
            t, ro = divmod(CHOFF[c], 128)
            for half in range(2):
                nc.sync.dma_start(
                    hyp_v[half * cs:(half + 1) * cs, :, c],
                    hyb_v[ro:ro + cs, t, half::2],
                )

        # ---- precompute hxb chunks = W1x @ x^T + b1 (bf16, dup rows) ----
        for t in (2, 0, 1):   # res chunk first (matches CORDER)
            hs = HT_SZ[t]
            ps = pp_pre.tile([128, N], f32, tag="pre")
            for k in range(KD):
                nc.tensor.matmul(
                    ps[0:hs, :],
                    lhsT=w1x_sb[:, k * H + 128 * t: k * H + 128 * t + hs],
                    rhs=xt_sb[:, k * N:(k + 1) * N],
                    start=(k == 0), stop=(k == KD - 1),
                )
            chunks = [(4, 0, 44)] if t == 2 else [(2 * t, 0, 64), (2 * t + 1, 64, 64)]
            for c, ro, cs in chunks:
                nc.scalar.activation(
                    hxd_sb[0:cs, c * N:(c + 1) * N], ps[ro:ro + cs, :],
                    AF.Identity, bias=b1_sb[ro:ro + cs, t:t + 1],
                )
                nc.sync.dma_start(
                    hxd_sb[cs:2 * cs, c * N:(c + 1) * N],
                    hxd_sb[0:cs, c * N:(c + 1) * N],
                )

        # ---- release precompute PSUM, allocate the 8 z banks ----
        pp_hy.release()
        pp_pre.release()
        zpool = tc.alloc_tile_pool(name="zb", bufs=1, space="PSUM")
        zb = [zpool.tile([128, N], f32, tag=f"zb{b}", name=f"zb{b}") for b in range(8)]

        # zero all banks (all-zeros matmul writes the full bank)
        for b in range(8):
            nc.tensor.matmul(
                zb[b][:], lhsT=zs_sb[:, 0:128], rhs=zs_sb[:, 0:N],
                start=True, stop=True,
            )

        # ---- main loop: 32 pairs x 5 chunks ----
        def make_tile(p, c, on_act):
            cs = CH[c]
            r = rpool.tile([128, N], bf16, tag="r")
            src = hxd_sb[0:2 * cs, c * N:(c + 1) * N]
            col = hyp_sb[0:2 * cs, 5 * p + c:5 * p + c + 1]
            if on_act:
                nc.scalar.activation(r[0:2 * cs, :], src, AF.Relu, bias=col)
            else:
                nc.vector.tensor_scalar(r[0:2 * cs, :], src, col, 0.0, ALU.add, ALU.max)
            return r

        for p in range(NPAIR):
            g = p % 4
            b = p // 4
            for c in CORDER:
                cs = CH[c]
                on_act = (c == 4) or (c == 0 and g == 0)
                r = make_tile(p, c, on_act)
                nc.tensor.matmul(
                    zb[b][32 * g:32 * g + 2, :],
                    lhsT=w2_sb[0:2 * cs, 2 * c:2 * c + 2],
                    rhs=r[0:2 * cs, :],
                    start=False, stop=(c == 3),
                    tile_position=(0, 32 * g),
                )
            if g == 3:
                # bank b complete: diag-col Exp, then full Exp + row sums
                nc.scalar.activation(
                    dee_sb[:, 8 * b:8 * b + 8], zb[b][:, 8 * b:8 * b + 8],
                    AF.Exp, bias=b2_sb[:],
                )
                ee = epool.tile([128, N], bf16, tag="ee")
                nc.scalar.activation(
                    ee[:], zb[b][:], AF.Exp, bias=b2_sb[:],
                    accum_out=acc_sb[:, b:b + 1],
                )

        # ---- tail ----
        onec = tpool.tile([128, 1], f32, tag="onec")
        nc.vector.memset(onec[:], 1.0)
        nnc = tpool.tile([128, 1], f32, tag="nnc")
        nc.vector.memset(nnc[:], float(N))
        t0ln = tpool.tile([128, 64], f32, tag="t0ln")
        nc.scalar.activation(t0ln[:], dee_sb[:], AF.Ln, bias=onec[:])
        t0m = tpool.tile([128, 64], f32, tag="t0m")
        nc.vector.tensor_tensor(t0m[:], t0ln[:], dmask_sb[:], ALU.mult)
        lsev = tpool.tile([128, 8], f32, tag="lsev")
        nc.scalar.activation(lsev[:], acc_sb[:], AF.Ln, bias=nnc[:])
        combo = tpool.tile([128, 2], f32, tag="combo")
        nc.vector.tensor_reduce(combo[:, 0:1], t0m[:], axis=AX.X, op=ALU.add)
        nc.vector.tensor_reduce(combo[:, 1:2], lsev[:], axis=AX.X, op=ALU.add)

        zpool.release()
        pp_tail = tc.alloc_tile_pool(name="pp_tail", bufs=1, space="PSUM")
        fps = pp_tail.tile([128, 2], f32, tag="fps")
        # cross-partition sum with the validity mask folded in as weights
        nc.tensor.matmul(
            fps[0:1, 0:2], lhsT=pmask_sb[:], rhs=combo[:],
            start=True, stop=True,
        )
        final = tpool.tile([1, 2], f32, tag="final")
        nc.vector.tensor_copy(final[:], fps[0:1, 0:2])
        nc.sync.dma_start(out[:], final[:])

        for pl in (pp_tail, tpool, epool, rpool, cpool):
            pl.release()

    nc.finalize()
    return nc


def _get_module():
    if "nc" not in _CACHE:
        _CACHE["nc"] = _build_module()
    return _CACHE["nc"]


def kernel(**inputs) -> np.ndarray:
    from concourse.bass_utils import run_bass_kernel_spmd

    x = np.ascontiguousarray(np.asarray(inputs["x_samples"], dtype=np.float32))
    y = np.ascontiguousarray(np.asarray(inputs["y_samples"], dtype=np.float32))
    W1 = np.asarray(inputs["W1"], dtype=np.float32)
    b1 = np.asarray(inputs["b1"], dtype=np.float32).reshape(H)
    W2 = np.asarray(inputs["W2"], dtype=np.float32)
    b2 = float(np.asarray(inputs["b2"], dtype=np.float32).reshape(1)[0])

    w1xT = np.ascontiguousarray(W1[:, :XD].T).astype(BF16)
    w1yT = np.ascontiguousarray(W1[:, XD:].T).astype(BF16)

    b1p = np.zeros((128, 3), np.float32)
    for t, hs in enumerate((128, 128, 44)):
        b1p[:hs, t] = b1[128 * t:128 * t + hs]

    w2 = W2.reshape(H)
    w2pk = np.zeros((128, 10), np.float32)
    for c in range(5):
        cs = CH[c]
        w2pk[0:cs, 2 * c] = w2[CHOFF[c]:CHOFF[c] + cs]
        w2pk[cs:2 * cs, 2 * c + 1] = w2[CHOFF[c]:CHOFF[c] + cs]
    w2pk = w2pk.astype(BF16)
    b2r = np.full((128, 1), b2, np.float32)

    # masks: valid slots are (partition 32g+h, bank b) -> i = 8b+2g+h
    dmask = np.zeros((128, 64), np.float32)
    pmask = np.zeros((128, 1), np.float32)
    for g in range(4):
        for h in range(2):
            pmask[32 * g + h, 0] = 1.0
            for b in range(8):
                dmask[32 * g + h, 8 * b + 2 * g + h] = 1.0

    in_maps = []
    for c in range(NCORES):
        sl = slice(c * ISH, (c + 1) * ISH)
        xrot = np.roll(x, -c * ISH, axis=0)          # diag of row i at col i
        in_maps.append({
            "xT": np.ascontiguousarray(xrot.T).astype(BF16),
            "w1xT": w1xT,
            "w1yT": w1yT,
            "yT": np.ascontiguousarray(y[sl].T).astype(BF16),
            "b1p": b1p,
            "w2pk": w2pk,
            "b2r": b2r,
            "dmask": dmask,
            "pmask": pmask,
        })

    nc = _get_module()
    res = run_bass_kernel_spmd(
        nc, in_maps, core_ids=list(range(NCORES)), trace=TRACE
    )
    global LAST_RESULTS
    LAST_RESULTS = res
    t0_sum = 0.0
    lse_sum = 0.0
    for r in res.results:
        o = r["out"]
        t0_sum += float(o[0, 0])
        lse_sum += float(o[0, 1])
    val = t0_sum / N - (lse_sum / N - math.log(N))
    return np.float32(val)


# revision 9
# speedup vs baseline: 1.8732x; 1.1078x over previous
"""InfoNCE lower-bound kernel for 8 Trainium2 NeuronCores (v2).

Math (reference):
  hx = x @ W1x.T ; hy = y @ W1y.T            [N, H]
  z_ij = relu(hx[j] + hy[i] + b1) . w2       (logit WITHOUT b2)
  T1[i,j] = softplus(z_ij + b2)
  T0[i]   = T1[i,i]
  lse[i]  = log(sum_j exp(T1[i,j])) = log(N + sum_j exp(z_ij + b2))
  out     = mean(T0) - (mean(lse) - log N)

Layout (per core, 64 i-rows):
  * bf16 on all wide paths; fp32 PSUM accumulation.
  * H=300 split into chunks (64,64,64,64,44). i-rows processed in PAIRS:
    one [128, 512] relu tile holds chunk rows for i (partitions 0..63)
    and i+1 (partitions 64..127), built by one DVE/Act op from a
    row-duplicated hxb copy plus a paired per-partition bias column.
  * w2 contraction = m=2 block-diagonal matvec (lhsT [128,2]) on the PE,
    column-tiled 4 ways (tile_position (0,32g)); pair p accumulates into
    PSUM rows {32g, 32g+1} of bank p//4 -- all 64 z-rows in flight.
  * Banks are zeroed by an all-zeros matmul first, so untouched
    partitions are exactly 0 (NaN-safe masking later).
  * Per-bank drain (overlapped): Act Exp of the 8 diag columns, then Act
    Exp of the full bank with accum_out row-sums.
  * x columns are rotated by 64*core so the T1 diagonal of row i lands
    at column i locally; T0 is read out of the grid itself.
"""

import math

import numpy as np
import ml_dtypes

N = 512
XD = 768
YD = 768
H = 300
NCORES = 8
ISH = N // NCORES   # 64 rows per core
KD = XD // 128      # 6 contraction tiles of 128
NPAIR = ISH // 2    # 32 pairs
CH = [64, 64, 64, 64, 44]        # h-chunk sizes
CHOFF = [0, 64, 128, 192, 256]   # h offset of each chunk
CORDER = [4, 0, 1, 2, 3]         # chunk issue order (res first, stop on c=3)
BF16 = ml_dtypes.bfloat16

_CACHE = {}
TRACE = False
LAST_RESULTS = None


def _build_module():
    import concourse.bacc as bacc
    import concourse.mybir as mybir
    from concourse.tile import TileContext

    f32 = mybir.dt.float32
    bf16 = mybir.dt.bfloat16
    AF = mybir.ActivationFunctionType
    ALU = mybir.AluOpType
    AX = mybir.AxisListType

    nc = bacc.Bacc("TRN2", target_bir_lowering=False, debug=False)

    xT = nc.dram_tensor("xT", [XD, N], bf16, kind="ExternalInput")      # x^T, cols rotated by 64*core
    w1xT = nc.dram_tensor("w1xT", [XD, H], bf16, kind="ExternalInput")
    w1yT = nc.dram_tensor("w1yT", [YD, H], bf16, kind="ExternalInput")
    yT = nc.dram_tensor("yT", [YD, ISH], bf16, kind="ExternalInput")    # y-slice^T
    b1p = nc.dram_tensor("b1p", [128, 3], f32, kind="ExternalInput")    # b1 packed by h-tile
    w2pk = nc.dram_tensor("w2pk", [128, 10], bf16, kind="ExternalInput")
    b2r = nc.dram_tensor("b2r", [128, 1], f32, kind="ExternalInput")
    dmask = nc.dram_tensor("dmask", [128, 64], f32, kind="ExternalInput")
    pmask = nc.dram_tensor("pmask", [128, 1], f32, kind="ExternalInput")
    out = nc.dram_tensor("out", [1, 2], f32, kind="ExternalOutput")     # [t0_sum, lse_sum]

    with TileContext(nc) as tc:
        cpool = tc.alloc_tile_pool(name="consts", bufs=1)
        rpool = tc.alloc_tile_pool(name="rtiles", bufs=16)
        tpool = tc.alloc_tile_pool(name="tail", bufs=1)
        pp_pre = tc.alloc_tile_pool(name="pp_pre", bufs=2, space="PSUM")
        pp_hy = tc.alloc_tile_pool(name="pp_hy", bufs=2, space="PSUM")

        # ---- persistent SBUF tiles ----
        xt_sb = cpool.tile([128, KD * N], bf16, tag="xt")
        w1x_sb = cpool.tile([128, KD * H], bf16, tag="w1x")
        w1y_sb = cpool.tile([128, KD * H], bf16, tag="w1y")
        yt_sb = cpool.tile([128, KD * ISH], bf16, tag="yt")
        b1_sb = cpool.tile([128, 3], f32, tag="b1")
        w2_sb = cpool.tile([128, 10], bf16, tag="w2")
        b2_sb = cpool.tile([128, 1], f32, tag="b2")
        dmask_sb = cpool.tile([128, 64], f32, tag="dmask")
        pmask_sb = cpool.tile([128, 1], f32, tag="pmask")
        hyb_sb = cpool.tile([128, 3 * ISH], f32, tag="hyb")    # hy + b1, per h-tile
        hyp_sb = cpool.tile([128, 5 * NPAIR], f32, tag="hyp")  # paired bias cols
        hxd_sb = cpool.tile([128, 5 * N], bf16, tag="hxd")     # row-duplicated hxb chunks
        dee_sb = cpool.tile([128, 64], f32, tag="dee")         # diag cols gathered from ee
        ee_sb = cpool.tile([128, 8 * N], bf16, tag="ee")       # e^(z+b2) per bank
        acc_sb = cpool.tile([128, 8], f32, tag="acc")          # row sums per bank
        zs_sb = cpool.tile([128, N], bf16, tag="zs")           # zero scratch

        # ---- DMA inputs ----
        nc.scalar.dma_start(
            w1y_sb[:].rearrange("p (k h) -> p k h", k=KD),
            w1yT[:].rearrange("(k p) h -> p k h", p=128))
        nc.scalar.dma_start(
            w1x_sb[:].rearrange("p (k h) -> p k h", k=KD),
            w1xT[:].rearrange("(k p) h -> p k h", p=128))
        nc.sync.dma_start(b1_sb[:], b1p[:])
        nc.sync.dma_start(w2_sb[:], w2pk[:])
        nc.sync.dma_start(b2_sb[:], b2r[:])
        nc.sync.dma_start(dmask_sb[:], dmask[:])
        nc.sync.dma_start(pmask_sb[:], pmask[:])
        nc.sync.dma_start(
            yt_sb[:].rearrange("p (k i) -> p k i", k=KD),
            yT[:].rearrange("(k p) i -> p k i", p=128))
        nc.sync.dma_start(
            xt_sb[:].rearrange("p (k n) -> p k n", k=KD),
            xT[:].rearrange("(k p) n -> p k n", p=128))

        # ---- PE warmup (HAM) on zeroed scratch during the DMA window ----
        nc.vector.memset(zs_sb[:], 0.0)
        onec = tpool.tile([128, 1], f32, tag="onec")
        nc.vector.memset(onec[:], 1.0)
        nnc = tpool.tile([128, 1], f32, tag="nnc")
        nc.vector.memset(nnc[:], float(N))
        # preload Exp/Ln activation tables off the critical path
        tdum = tpool.tile([1, 2], f32, tag="tdum")
        nc.scalar.activation(tdum[0:1, 0:1], onec[0:1, :], AF.Exp, bias=onec[0:1, :])
        nc.scalar.activation(tdum[0:1, 1:2], onec[0:1, :], AF.Ln, bias=onec[0:1, :])
        for w in range(24):
            wps = pp_pre.tile([128, N], f32, tag="pre")
            nc.tensor.matmul(
                wps[0:128, 0:128], lhsT=zs_sb[:, 0:128], rhs=zs_sb[:, 0:128],
                start=True, stop=True,
            )

        # ---- precompute hy' = y @ W1y^T + b1 (fp32, 3 h-tiles) ----
        HT_SZ = [128, 128, 44]
        for t in range(3):
            hs = HT_SZ[t]
            psy = pp_hy.tile([128, ISH], f32, tag="hy")
            for k in range(KD):
                nc.tensor.matmul(
                    psy[0:hs, :],
                    lhsT=w1y_sb[:, k * H + 128 * t: k * H + 128 * t + hs],
                    rhs=yt_sb[:, k * ISH:(k + 1) * ISH],
                    start=(k == 0), stop=(k == KD - 1),
                )
            nc.vector.tensor_scalar(
                hyb_sb[0:hs, t * ISH:(t + 1) * ISH], psy[0:hs, :],
                b1_sb[0:hs, t:t + 1], None, ALU.add,
            )

        # paired bias columns: hyp[0:cs, 5p+c] = hy'[chunk c, 2p],
        #                      hyp[cs:2cs, 5p+c] = hy'[chunk c, 2p+1]
        hyb_v = hyb_sb[:].rearrange("p (t i) -> p t i", t=3)
        hyp_v = hyp_sb[:].rearrange("p (q c) -> p q c", c=5)
        for c in range(5):
            cs = CH[c]
            t, ro = divmod(CHOFF[c], 128)
            for half in range(2):
                nc.sync.dma_start(
                    hyp_v[half * cs:(half + 1) * cs, :, c],
                    hyb_v[ro:ro + cs, t, half::2],
                )

        # ---- precompute hxb chunks = W1x @ x^T + b1 (bf16, dup rows) ----
        for t in (2, 0, 1):   # res chunk first (matches CORDER)
            hs = HT_SZ[t]
            ps = pp_pre.tile([128, N], f32, tag="pre")
            for k in range(KD):
                nc.tensor.matmul(
                    ps[0:hs, :],
                    lhsT=w1x_sb[:, k * H + 128 * t: k * H + 128 * t + hs],
                    rhs=xt_sb[:, k * N:(k + 1) * N],
                    start=(k == 0), stop=(k == KD - 1),
                )
            chunks = [(4, 0, 44)] if t == 2 else [(2 * t, 0, 64), (2 * t + 1, 64, 64)]
            for c, ro, cs in chunks:
                nc.scalar.activation(
                    hxd_sb[0:cs, c * N:(c + 1) * N], ps[ro:ro + cs, :],
                    AF.Identity, bias=b1_sb[ro:ro + cs, t:t + 1],
                )
                nc.sync.dma_start(
                    hxd_sb[cs:2 * cs, c * N:(c + 1) * N],
                    hxd_sb[0:cs, c * N:(c + 1) * N],
                )

        # ---- release precompute PSUM, allocate the 8 z banks ----
        pp_hy.release()
        pp_pre.release()
        zpool = tc.alloc_tile_pool(name="zb", bufs=1, space="PSUM")
        zb = [zpool.tile([128, N], f32, tag=f"zb{b}", name=f"zb{b}") for b in range(8)]

        # zero all banks (all-zeros matmul writes the full bank)
        for b in range(8):
            nc.tensor.matmul(
                zb[b][:], lhsT=zs_sb[:, 0:128], rhs=zs_sb[:, 0:N],
                start=True, stop=True,
            )

        # ---- main loop: 32 pairs x 5 chunks ----
        def make_tile(p, c, on_act):
            cs = CH[c]
            r = rpool.tile([128, N], bf16, tag="r")
            src = hxd_sb[0:2 * cs, c * N:(c + 1) * N]
            col = hyp_sb[0:2 * cs, 5 * p + c:5 * p + c + 1]
            if on_act:
                nc.scalar.activation(r[0:2 * cs, :], src, AF.Relu, bias=col)
            else:
                nc.vector.tensor_scalar(r[0:2 * cs, :], src, col, 0.0, ALU.add, ALU.max)
            return r

        for p in range(NPAIR):
            g = p % 4
            b = p // 4
            for c in CORDER:
                cs = CH[c]
                on_act = (c == 4) or (c == 0 and p in (0, 16))
                r = make_tile(p, c, on_act)
                nc.tensor.matmul(
                    zb[b][32 * g:32 * g + 2, :],
                    lhsT=w2_sb[0:2 * cs, 2 * c:2 * c + 2],
                    rhs=r[0:2 * cs, :],
                    start=False, stop=(c == 3),
                    tile_position=(0, 32 * g),
                )
            if g == 3:
                # bank b complete: full Exp + row sums, diag cols via DVE copy
                nc.scalar.activation(
                    ee_sb[:, b * N:(b + 1) * N], zb[b][:], AF.Exp, bias=b2_sb[:],
                    accum_out=acc_sb[:, b:b + 1],
                )
                nc.vector.tensor_copy(
                    dee_sb[:, 8 * b:8 * b + 8],
                    ee_sb[:, b * N + 8 * b:b * N + 8 * b + 8],
                )

        # ---- tail ----
        t0ln = tpool.tile([128, 64], f32, tag="t0ln")
        nc.scalar.activation(t0ln[:], dee_sb[:], AF.Ln, bias=onec[:])
        t0m = tpool.tile([128, 64], f32, tag="t0m")
        nc.vector.tensor_tensor(t0m[:], t0ln[:], dmask_sb[:], ALU.mult)
        lsev = tpool.tile([128, 8], f32, tag="lsev")
        nc.scalar.activation(lsev[:], acc_sb[:], AF.Ln, bias=nnc[:])
        combo = tpool.tile([128, 2], f32, tag="combo")
        nc.vector.tensor_reduce(combo[:, 0:1], t0m[:], axis=AX.X, op=ALU.add)
        nc.vector.tensor_reduce(combo[:, 1:2], lsev[:], axis=AX.X, op=ALU.add)

        zpool.release()
        pp_tail = tc.alloc_tile_pool(name="pp_tail", bufs=1, space="PSUM")
        fps = pp_tail.tile([128, 2], f32, tag="fps")
        # cross-partition sum with the validity mask folded in as weights
        nc.tensor.matmul(
            fps[0:1, 0:2], lhsT=pmask_sb[:], rhs=combo[:],
            start=True, stop=True,
        )
        final = tpool.tile([1, 2], f32, tag="final")
        nc.vector.tensor_copy(final[:], fps[0:1, 0:2])
        nc.sync.dma_start(out[:], final[:])

        for pl in (pp_tail, tpool, rpool, cpool):
            pl.release()

    nc.finalize()
    return nc


def _get_module():
    if "nc" not in _CACHE:
        _CACHE["nc"] = _build_module()
    return _CACHE["nc"]


def kernel(**inputs) -> np.ndarray:
    from concourse.bass_utils import run_bass_kernel_spmd

    x = np.ascontiguousarray(np.asarray(inputs["x_samples"], dtype=np.float32))
    y = np.ascontiguousarray(np.asarray(inputs["y_samples"], dtype=np.float32))
    W1 = np.asarray(inputs["W1"], dtype=np.float32)
    b1 = np.asarray(inputs["b1"], dtype=np.float32).reshape(H)
    W2 = np.asarray(inputs["W2"], dtype=np.float32)
    b2 = float(np.asarray(inputs["b2"], dtype=np.float32).reshape(1)[0])

    w1xT = np.ascontiguousarray(W1[:, :XD].T).astype(BF16)
    w1yT = np.ascontiguousarray(W1[:, XD:].T).astype(BF16)

    b1p = np.zeros((128, 3), np.float32)
    for t, hs in enumerate((128, 128, 44)):
        b1p[:hs, t] = b1[128 * t:128 * t + hs]

    w2 = W2.reshape(H)
    w2pk = np.zeros((128, 10), np.float32)
    for c in range(5):
        cs = CH[c]
        w2pk[0:cs, 2 * c] = w2[CHOFF[c]:CHOFF[c] + cs]
        w2pk[cs:2 * cs, 2 * c + 1] = w2[CHOFF[c]:CHOFF[c] + cs]
    w2pk = w2pk.astype(BF16)
    b2r = np.full((128, 1), b2, np.float32)

    # masks: valid slots are (partition 32g+h, bank b) -> i = 8b+2g+h
    dmask = np.zeros((128, 64), np.float32)
    pmask = np.zeros((128, 1), np.float32)
    for g in range(4):
        for h in range(2):
            pmask[32 * g + h, 0] = 1.0
            for b in range(8):
                dmask[32 * g + h, 8 * b + 2 * g + h] = 1.0

    in_maps = []
    for c in range(NCORES):
        sl = slice(c * ISH, (c + 1) * ISH)
        xrot = np.roll(x, -c * ISH, axis=0)          # diag of row i at col i
        in_maps.append({
            "xT": np.ascontiguousarray(xrot.T).astype(BF16),
            "w1xT": w1xT,
            "w1yT": w1yT,
            "yT": np.ascontiguousarray(y[sl].T).astype(BF16),
            "b1p": b1p,
            "w2pk": w2pk,
            "b2r": b2r,
            "dmask": dmask,
            "pmask": pmask,
        })

    nc = _get_module()
    res = run_bass_kernel_spmd(
        nc, in_maps, core_ids=list(range(NCORES)), trace=TRACE
    )
    global LAST_RESULTS
    LAST_RESULTS = res
    t0_sum = 0.0
    lse_sum = 0.0
    for r in res.results:
        o = r["out"]
        t0_sum += float(o[0, 0])
        lse_sum += float(o[0, 1])
    val = t0_sum / N - (lse_sum / N - math.log(N))
    return np.float32(val)


# revision 11
# speedup vs baseline: 2.1178x; 1.1306x over previous
"""InfoNCE lower-bound kernel for 8 Trainium2 NeuronCores (v2).

Math (reference):
  hx = x @ W1x.T ; hy = y @ W1y.T            [N, H]
  z_ij = relu(hx[j] + hy[i] + b1) . w2       (logit WITHOUT b2)
  T1[i,j] = softplus(z_ij + b2)
  T0[i]   = T1[i,i]
  lse[i]  = log(sum_j exp(T1[i,j])) = log(N + sum_j exp(z_ij + b2))
  out     = mean(T0) - (mean(lse) - log N)

Layout (per core, 64 i-rows):
  * bf16 on all wide paths; fp32 PSUM accumulation.
  * H=300 split into chunks (64,64,64,64,44). i-rows processed in PAIRS:
    one [128, 512] relu tile holds chunk rows for i (partitions 0..63)
    and i+1 (partitions 64..127), built by one DVE/Act op from a
    row-duplicated hxb copy plus a paired per-partition bias column.
  * w2 contraction = m=2 block-diagonal matvec (lhsT [128,2]) on the PE,
    column-tiled 4 ways (tile_position (0,32g)); pair p accumulates into
    PSUM rows {32g, 32g+1} of bank p//4 -- all 64 z-rows in flight.
  * Banks are zeroed by an all-zeros matmul first, so untouched
    partitions are exactly 0 (NaN-safe masking later).
  * Per-bank drain (overlapped): Act Exp of the 8 diag columns, then Act
    Exp of the full bank with accum_out row-sums.
  * x columns are rotated by 64*core so the T1 diagonal of row i lands
    at column i locally; T0 is read out of the grid itself.
"""

import math

import numpy as np
import ml_dtypes

N = 512
XD = 768
YD = 768
H = 300
NCORES = 8
ISH = N // NCORES   # 64 rows per core
KD = XD // 128      # 6 contraction tiles of 128
NPAIR = ISH // 2    # 32 pairs
CH = [64, 64, 64, 64, 44]        # h-chunk sizes
CHOFF = [0, 64, 128, 192, 256]   # h offset of each chunk
CORDER = [4, 0, 1, 2, 3]         # chunk issue order (res first, stop on c=3)
BF16 = ml_dtypes.bfloat16

_CACHE = {}
TRACE = False
LAST_RESULTS = None


def _build_module():
    import concourse.bacc as bacc
    import concourse.mybir as mybir
    from concourse.tile import TileContext

    f32 = mybir.dt.float32
    bf16 = mybir.dt.bfloat16
    AF = mybir.ActivationFunctionType
    ALU = mybir.AluOpType
    AX = mybir.AxisListType

    nc = bacc.Bacc("TRN2", target_bir_lowering=False, debug=False)

    xT = nc.dram_tensor("xT", [XD, N], bf16, kind="ExternalInput")      # x^T, cols rotated by 64*core
    w1xT = nc.dram_tensor("w1xT", [XD, H], bf16, kind="ExternalInput")
    w1yT = nc.dram_tensor("w1yT", [YD, H], bf16, kind="ExternalInput")
    yT = nc.dram_tensor("yT", [YD, ISH], bf16, kind="ExternalInput")    # y-slice^T
    b1p = nc.dram_tensor("b1p", [128, 3], f32, kind="ExternalInput")    # b1 packed by h-tile
    w2pk = nc.dram_tensor("w2pk", [128, 10], bf16, kind="ExternalInput")
    b2r = nc.dram_tensor("b2r", [128, 1], f32, kind="ExternalInput")
    dmask = nc.dram_tensor("dmask", [128, 64], f32, kind="ExternalInput")
    pmask = nc.dram_tensor("pmask", [128, 1], f32, kind="ExternalInput")
    out = nc.dram_tensor("out", [1, 2], f32, kind="ExternalOutput")     # [t0_sum, lse_sum]

    with TileContext(nc) as tc:
        cpool = tc.alloc_tile_pool(name="consts", bufs=1)
        rpool = tc.alloc_tile_pool(name="rtiles", bufs=16)
        tpool = tc.alloc_tile_pool(name="tail", bufs=1)
        zpool = tc.alloc_tile_pool(name="zb", bufs=1, space="PSUM")

        # ---- persistent SBUF tiles ----
        xt_sb = cpool.tile([128, KD * N], bf16, tag="xt")
        w1x_sb = cpool.tile([128, KD * H], bf16, tag="w1x")
        w1y_sb = cpool.tile([128, KD * H], bf16, tag="w1y")
        yt_sb = cpool.tile([128, KD * ISH], bf16, tag="yt")
        b1_sb = cpool.tile([128, 3], f32, tag="b1")
        w2_sb = cpool.tile([128, 10], bf16, tag="w2")
        b2_sb = cpool.tile([128, 1], f32, tag="b2")
        dmask_sb = cpool.tile([128, 64], f32, tag="dmask")
        pmask_sb = cpool.tile([128, 1], f32, tag="pmask")
        hyp_sb = cpool.tile([128, 5 * NPAIR], f32, tag="hyp")  # paired bias cols
        hxd_sb = cpool.tile([128, 5 * N], bf16, tag="hxd")     # row-duplicated hxb chunks
        dee_sb = cpool.tile([128, 64], f32, tag="dee")         # diag cols gathered from ee
        ee_sb = cpool.tile([128, 8 * N], bf16, tag="ee")       # e^(z+b2) per bank
        acc_sb = cpool.tile([128, 8], f32, tag="acc")          # row sums per bank
        zs_sb = cpool.tile([128, N], bf16, tag="zs")           # zero scratch

        # ---- DMA inputs ----
        nc.scalar.dma_start(
            w1y_sb[:].rearrange("p (k h) -> p k h", k=KD),
            w1yT[:].rearrange("(k p) h -> p k h", p=128))
        nc.scalar.dma_start(
            w1x_sb[:].rearrange("p (k h) -> p k h", k=KD),
            w1xT[:].rearrange("(k p) h -> p k h", p=128))
        nc.scalar.dma_start(b1_sb[:], b1p[:])
        nc.scalar.dma_start(w2_sb[:], w2pk[:])
        nc.scalar.dma_start(b2_sb[:], b2r[:])
        nc.scalar.dma_start(dmask_sb[:], dmask[:])
        nc.scalar.dma_start(pmask_sb[:], pmask[:])
        nc.sync.dma_start(
            yt_sb[:].rearrange("p (k i) -> p k i", k=KD),
            yT[:].rearrange("(k p) i -> p k i", p=128))
        nc.sync.dma_start(
            xt_sb[:].rearrange("p (k n) -> p k n", k=KD),
            xT[:].rearrange("(k p) n -> p k n", p=128))

        zb = [zpool.tile([128, N], f32, tag=f"zb{b}", name=f"zb{b}") for b in range(8)]

        # ---- PE warmup (HAM) on zeroed scratch during the DMA window ----
        nc.vector.memset(zs_sb[:], 0.0)
        nc.vector.memset(hxd_sb[:, 4 * N:5 * N], 0.0)   # res chunk gap rows
        nc.vector.memset(hyp_sb[:], 0.0)
        onec = tpool.tile([128, 1], f32, tag="onec")
        nc.vector.memset(onec[:], 1.0)
        nnc = tpool.tile([128, 1], f32, tag="nnc")
        nc.vector.memset(nnc[:], float(N))
        # preload Exp/Ln activation tables off the critical path
        tdum = tpool.tile([1, 2], f32, tag="tdum")
        nc.scalar.activation(tdum[0:1, 0:1], onec[0:1, :], AF.Exp, bias=onec[0:1, :])
        nc.scalar.activation(tdum[0:1, 1:2], onec[0:1, :], AF.Ln, bias=onec[0:1, :])
        for w in range(56):
            nc.tensor.matmul(
                zb[3][0:1, 0:64], lhsT=zs_sb[:, 0:1], rhs=zs_sb[:, 0:64],
                start=True, stop=True,
            )

        # ---- precompute hy = y @ W1y^T into z banks 0..2; drain (+b1)
        #      straight into the paired-column layout; then zero the bank ----
        HT_SZ = [128, 128, 44]
        hyp_v = hyp_sb[:].rearrange("p (q c) -> p q c", c=5)
        CH_BY_T = {0: [(0, 0, 64), (1, 64, 64)], 1: [(2, 0, 64), (3, 64, 64)],
                   2: [(4, 0, 44)]}
        for t in range(3):
            hs = HT_SZ[t]
            for k in range(KD):
                nc.tensor.matmul(
                    zb[t][0:hs, 0:ISH],
                    lhsT=w1y_sb[:, k * H + 128 * t: k * H + 128 * t + hs],
                    rhs=yt_sb[:, k * ISH:(k + 1) * ISH],
                    start=(k == 0), stop=(k == KD - 1),
                )
            for c, ro, cs in CH_BY_T[t]:
                hb = 64 if c == 4 else cs
                for half in range(2):
                    nc.vector.tensor_scalar(
                        hyp_v[half * hb:half * hb + cs, :, c],
                        zb[t][ro:ro + cs, 0:ISH][:, half::2],
                        b1_sb[ro:ro + cs, t:t + 1], None, ALU.add,
                    )
            nc.vector.memset(zb[t][:], 0.0)

        # ---- precompute hxb chunks = W1x @ x^T + b1 into zb[5..7] ----
        HXB_BANK = {2: 7, 0: 5, 1: 6}
        for t in (2, 0, 1):   # res chunk first (matches CORDER)
            hs = HT_SZ[t]
            bnk = HXB_BANK[t]
            for k in range(KD):
                nc.tensor.matmul(
                    zb[bnk][0:hs, :],
                    lhsT=w1x_sb[:, k * H + 128 * t: k * H + 128 * t + hs],
                    rhs=xt_sb[:, k * N:(k + 1) * N],
                    start=(k == 0), stop=(k == KD - 1),
                )
            for c, ro, cs in CH_BY_T[t]:
                nc.scalar.activation(
                    hxd_sb[0:cs, c * N:(c + 1) * N], zb[bnk][ro:ro + cs, :],
                    AF.Identity, bias=b1_sb[ro:ro + cs, t:t + 1],
                )
                hb = 64 if c == 4 else cs
                nc.vector.tensor_copy(
                    hxd_sb[hb:hb + cs, c * N:(c + 1) * N],
                    hxd_sb[0:cs, c * N:(c + 1) * N],
                )
            nc.vector.memset(zb[bnk][:], 0.0)
        nc.vector.memset(zb[3][:], 0.0)
        nc.vector.memset(zb[4][:], 0.0)

        # ---- main loop: 32 pairs x 5 chunks ----
        def make_tile(p, c, on_act):
            cs = 128 if c == 4 else 2 * CH[c]
            r = rpool.tile([128, N], bf16, tag="r")
            src = hxd_sb[0:cs, c * N:(c + 1) * N]
            col = hyp_sb[0:cs, 5 * p + c:5 * p + c + 1]
            if on_act:
                nc.scalar.activation(r[0:cs, :], src, AF.Relu, bias=col)
            else:
                nc.vector.tensor_scalar(r[0:cs, :], src, col, 0.0, ALU.add, ALU.max)
            return r

        for p in range(NPAIR):
            g = p % 4
            b = p // 4
            for c in CORDER:
                cs = 128 if c == 4 else 2 * CH[c]
                on_act = (c == 4) or (c == 0 and p % 5 == 2)
                r = make_tile(p, c, on_act)
                nc.tensor.matmul(
                    zb[b][32 * g:32 * g + 2, :],
                    lhsT=w2_sb[0:cs, 2 * c:2 * c + 2],
                    rhs=r[0:cs, :],
                    start=False, stop=(c == 3),
                    tile_position=(0, 32 * g),
                )
            if g == 3:
                # bank b complete: full Exp + row sums, diag cols via DVE copy
                nc.scalar.activation(
                    ee_sb[:, b * N:(b + 1) * N], zb[b][:], AF.Exp, bias=b2_sb[:],
                    accum_out=acc_sb[:, b:b + 1],
                )
                nc.vector.tensor_copy(
                    dee_sb[:, 8 * b:8 * b + 8],
                    ee_sb[:, b * N + 8 * b:b * N + 8 * b + 8],
                )

        # ---- tail ----
        t0ln = tpool.tile([128, 64], f32, tag="t0ln")
        nc.scalar.activation(t0ln[:], dee_sb[:], AF.Ln, bias=onec[:])
        t0m = tpool.tile([128, 64], f32, tag="t0m")
        nc.vector.tensor_tensor(t0m[:], t0ln[:], dmask_sb[:], ALU.mult)
        lsev = tpool.tile([128, 8], f32, tag="lsev")
        nc.scalar.activation(lsev[:], acc_sb[:], AF.Ln, bias=nnc[:])
        combo = tpool.tile([128, 2], f32, tag="combo")
        nc.vector.tensor_reduce(combo[:, 0:1], t0m[:], axis=AX.X, op=ALU.add)
        nc.vector.tensor_reduce(combo[:, 1:2], lsev[:], axis=AX.X, op=ALU.add)

        zpool.release()
        pp_tail = tc.alloc_tile_pool(name="pp_tail", bufs=1, space="PSUM")
        fps = pp_tail.tile([128, 2], f32, tag="fps")
        # cross-partition sum with the validity mask folded in as weights
        nc.tensor.matmul(
            fps[0:1, 0:2], lhsT=pmask_sb[:], rhs=combo[:],
            start=True, stop=True,
        )
        final = tpool.tile([1, 2], f32, tag="final")
        nc.vector.tensor_copy(final[:], fps[0:1, 0:2])
        nc.sync.dma_start(out[:], final[:])

        for pl in (pp_tail, tpool, rpool, cpool):
            pl.release()

    nc.finalize()
    return nc


def _get_module():
    if "nc" not in _CACHE:
        _CACHE["nc"] = _build_module()
    return _CACHE["nc"]


def kernel(**inputs) -> np.ndarray:
    from concourse.bass_utils import run_bass_kernel_spmd

    x = np.ascontiguousarray(np.asarray(inputs["x_samples"], dtype=np.float32))
    y = np.ascontiguousarray(np.asarray(inputs["y_samples"], dtype=np.float32))
    W1 = np.asarray(inputs["W1"], dtype=np.float32)
    b1 = np.asarray(inputs["b1"], dtype=np.float32).reshape(H)
    W2 = np.asarray(inputs["W2"], dtype=np.float32)
    b2 = float(np.asarray(inputs["b2"], dtype=np.float32).reshape(1)[0])

    w1xT = np.ascontiguousarray(W1[:, :XD].T).astype(BF16)
    w1yT = np.ascontiguousarray(W1[:, XD:].T).astype(BF16)

    b1p = np.zeros((128, 3), np.float32)
    for t, hs in enumerate((128, 128, 44)):
        b1p[:hs, t] = b1[128 * t:128 * t + hs]

    w2 = W2.reshape(H)
    w2pk = np.zeros((128, 10), np.float32)
    for c in range(5):
        cs = CH[c]
        hb = 64 if c == 4 else cs          # second-half base (32-aligned)
        w2pk[0:cs, 2 * c] = w2[CHOFF[c]:CHOFF[c] + cs]
        w2pk[hb:hb + cs, 2 * c + 1] = w2[CHOFF[c]:CHOFF[c] + cs]
    w2pk = w2pk.astype(BF16)
    b2r = np.full((128, 1), b2, np.float32)

    # masks: valid slots are (partition 32g+h, bank b) -> i = 8b+2g+h
    dmask = np.zeros((128, 64), np.float32)
    pmask = np.zeros((128, 1), np.float32)
    for g in range(4):
        for h in range(2):
            pmask[32 * g + h, 0] = 1.0
            for b in range(8):
                dmask[32 * g + h, 8 * b + 2 * g + h] = 1.0

    in_maps = []
    for c in range(NCORES):
        sl = slice(c * ISH, (c + 1) * ISH)
        xrot = np.roll(x, -c * ISH, axis=0)          # diag of row i at col i
        in_maps.append({
            "xT": np.ascontiguousarray(xrot.T).astype(BF16),
            "w1xT": w1xT,
            "w1yT": w1yT,
            "yT": np.ascontiguousarray(y[sl].T).astype(BF16),
            "b1p": b1p,
            "w2pk": w2pk,
            "b2r": b2r,
            "dmask": dmask,
            "pmask": pmask,
        })

    nc = _get_module()
    res = run_bass_kernel_spmd(
        nc, in_maps, core_ids=list(range(NCORES)), trace=TRACE
    )
    global LAST_RESULTS
    LAST_RESULTS = res
    t0_sum = 0.0
    lse_sum = 0.0
    for r in res.results:
        o = r["out"]
        t0_sum += float(o[0, 0])
        lse_sum += float(o[0, 1])
    val = t0_sum / N - (lse_sum / N - math.log(N))
    return np.float32(val)
